# revision 67
# baseline (speedup 1.0000x reference)
"""TRN2 Bass kernel for nn_CMAT_4561255269047 (dual-stream CNN + cross-attention).

Data-parallel over batch B=8 across 8 NeuronCores (1 sample/core, no collectives).

Per-core program (all matmuls fp32r at full PE rate):
  conv3x3 = 9 shifted matmuls over zero-padded [C,46,46] images, accumulated in
  PSUM over input-channel chunks (ci-outer loop, 8 PSUM banks resident).
  conv1 -> BN+ReLU fused into the PSUM-drain activation (scale/bias APs).
  conv2 -> gated residual relu((o2w+b)*o1 + (o2b+b)) via scalar_tensor_tensor.
  attention: sT[n,m] = k^T q (K=32), eT = exp(sT) (scores are small, no max
  subtraction), feat[c,m] = vT^T @ eT, Z[m] via ones-column matmul, normalize
  by 1/Z broadcast through a K=1 matmul, residual add, DMA out per chunk.
  gate*beta / (1-gate)*gamma are folded into vw/vb on the host.
"""
import sys
sys.path.insert(0, '/opt/trn_rl_repo')

import numpy as np
import ml_dtypes

import concourse.bass as bass
import concourse.mybir as mybir
import concourse.tile as tile
from concourse import bacc
from concourse.bass_utils import run_bass_kernel_spmd

MM_KINDS = {}

F32 = mybir.dt.float32
F32R = mybir.dt.float32r
BF16 = mybir.dt.bfloat16
BF16_CONV = True  # bf16 convs save ~27us but cost 12x accuracy (6e-3 vs 5e-4)
CONV_DT = BF16 if BF16_CONV else F32R
EPS = 1e-5
AF = mybir.ActivationFunctionType
ALU = mybir.AluOpType

H = W = 44
HP = WP = 46
N = H * W            # 1936
NCH = 4              # spatial n-chunks of 11 rows (484 px) for convs / att m
ROWS = 11
PX = ROWS * W        # 484
AJ = 16              # attention n-chunks of 128 (last = 16)

# prm packed-param columns
C_BNS1, C_BNT1, C_BNS2, C_BNT2 = 0, 2, 4, 6
C_C2B1, C_C2B2 = 8, 12
C_QB1, C_KB1, C_QB2, C_KB2 = 16, 17, 18, 19
C_VB1, C_VB2 = 20, 22            # v-bias as per-partition scalars, 2 c-chunks each
C_ONESR, C_ONESC = 24, 152       # ones row (partition 0) / ones column
C_ZERO = 153                     # 46 zero cols (o1p border source)
C_ONESP = 199                    # partial ones column (rows 0:16) for the last z chunk
C_ZEROW = 200                    # 484 zero cols (K-padding source)
PRM_COLS = 684
NK = 2048                        # k4 columns: N padded so jn=15 runs full M=128


def _mm(nc, kind, *args, **kw):
    inst = nc.tensor.matmul(*args, **kw)
    try:
        MM_KINDS[inst.ins.name] = kind
    except Exception:
        pass
    return inst


def _conv_stream(nc, tc, x_d, w1_d, w2_d, bns_col, bnt_col, c2b_col,
                 prm_t, o1p_t, out_t, wpool, xpool, cps, ctmp, zero_borders,
                 mid_hook=None, pre_x=None):
    """One sa_block: conv1 (1D-Winograd F(2,3) along W) -> BN+relu -> o1p_t
    (padded, normal layout via stride-2 stores), conv2 + gating -> out_t.

    x_d arrives in even/odd-split layout: cols 0:23 = even padded cols,
    23:46 = odd. w1_d is G-transformed on the host: [ci, 128, 3dy*4wc, 256].
    """
    f32 = lambda ap: ap.bitcast(F32)

    if zero_borders:
        # zero the o1p padding ring once (interior is fully overwritten per stream)
        zsrc = prm_t[:, C_ZERO:C_ZERO + HP]
        for ci in range(2):
            nc.vector.tensor_copy(o1p_t[:, ci, 0, :], zsrc)
            nc.vector.tensor_copy(o1p_t[:, ci, HP - 1, :], zsrc)
            nc.vector.tensor_copy(o1p_t[:, ci, :, 0], zsrc)
            nc.vector.tensor_copy(o1p_t[:, ci, :, HP - 1], zsrc)

    # ---- conv1 Winograd: Cin=512 (4 resident ci chunks) -> C=256 (2 m chunks),
    # rows in 2 halves of 22, w in 22 tiles of 2 outputs ----
    xs = []
    WT = 22  # w tiles / output cols per parity

    def _wdma_transform(nh, ci):
        w1c = wpool.tile([128, 12, 256], CONV_DT, tag="w1g", bufs=3,
                         name=f"w1g{nh}_{ci}")
        # dy=0 taps first: the first matmuls wait on only 1/3 of the transfer
        nc.sync.dma_start(w1c[:, 0:4, :], w1_d[ci][:, 0:4, :])
        nc.sync.dma_start(w1c[:, 4:12, :], w1_d[ci][:, 4:12, :])
        R = slice(22 * nh, 22 * nh + 24)
        xpc = xs[ci]
        xt = ctmp.tile([128, 4, 24, WT], CONV_DT, tag="xt", bufs=3, name=f"xt{nh}_{ci}")
        # B^T d per w-tile: d = [ev_j, od_j, ev_j1, od_j1]
        nc.vector.tensor_tensor(xt[:, 0], xpc[:, R, 0:22], xpc[:, R, 1:23], ALU.subtract)
        nc.vector.tensor_tensor(xt[:, 1], xpc[:, R, 23:45], xpc[:, R, 1:23], ALU.add)
        nc.vector.tensor_tensor(xt[:, 2], xpc[:, R, 1:23], xpc[:, R, 23:45], ALU.subtract)
        nc.vector.tensor_tensor(xt[:, 3], xpc[:, R, 23:45], xpc[:, R, 24:46], ALU.subtract)
        return w1c, xt

    pre = {}
    c2pre = {}
    for ci in range(4):
        if ci == 0 and pre_x is not None:
            # ci0's x tile + weight + transform were prefetched by the previous
            # stream's mid-hook (they ran under its conv2 matmuls)
            xs.append(pre_x['xpc'])
            pre[(0, 0)] = (pre_x['w1c'], pre_x['xt'])
            continue
        xpc = xpool.tile([128, HP, WP], CONV_DT, tag="xpad", bufs=4, name=f"xpad{ci}")
        nc.sync.dma_start(xpc[:, 0:24, :], x_d[ci][:, 0:24, :])
        nc.sync.dma_start(xpc[:, 24:HP, :], x_d[ci][:, 24:HP, :])
        xs.append(xpc)
        if ci == 0:
            # ci0's weight DMA + transform go right behind ci0's x transfer so
            # the first matmul isn't queued behind 6 unrelated DMAs
            pre[(0, 0)] = _wdma_transform(0, 0)
    for nh in range(2):
        r0 = 22 * nh
        ps = {}
        for mch in range(2):
            for wc in range(4):
                ps[(mch, wc)] = cps.tile([128, WT * WT], F32, tag="cps",
                                         name=f"c1p_{nh}_{mch}_{wc}")
        for ci in range(4):
            w1c, xt = pre.pop((nh, ci), None) or _wdma_transform(nh, ci)
            for mch in range(2):
                for dy in range(3):
                    for wc in range(4):
                        _mm(nc, "conv1",
                            ps[(mch, wc)][:].rearrange("p (a b) -> p a b", a=WT),
                            w1c[:, 4 * dy + wc, 128 * mch:128 * (mch + 1)],
                            xt[:, wc, dy:dy + WT, :],
                            start=(ci == 0 and dy == 0),
                            stop=(ci == 3 and dy == 2),
                            skip_group_check=True)
        if nh == 0:
            # prefetch next half's transforms ahead of the inverse ops in the
            # DVE queue so the nh=1 matmuls aren't blocked behind them
            pre[(1, 0)] = _wdma_transform(1, 0)
            pre[(1, 1)] = _wdma_transform(1, 1)
        for mch in range(2):
            m0, m1, m2, m3 = (ps[(mch, wc)][:].rearrange("p (a b) -> p a b", a=WT)
                              for wc in range(4))
            s = ctmp.tile([128, WT, WT], CONV_DT, tag="iv", bufs=6, name=f"s{nh}_{mch}")
            nc.scalar.copy(s[:], m1)
            a0 = ctmp.tile([128, WT, WT], CONV_DT, tag="iv", bufs=6, name=f"a0{nh}_{mch}")
            nc.vector.tensor_tensor(a0[:], m2, s[:], ALU.add)
            z0 = ctmp.tile([128, WT, WT], CONV_DT, tag="iv", bufs=6, name=f"z0{nh}_{mch}")
            nc.vector.tensor_tensor(z0[:], m0, a0[:], ALU.add)
            a1 = ctmp.tile([128, WT, WT], CONV_DT, tag="iv", bufs=6, name=f"a1{nh}_{mch}")
            nc.vector.tensor_tensor(a1[:], s[:], m2, ALU.subtract)
            z1 = ctmp.tile([128, WT, WT], CONV_DT, tag="iv", bufs=6, name=f"z1{nh}_{mch}")
            nc.vector.tensor_tensor(z1[:], a1[:], m3, ALU.subtract)
            # o1 = relu(z * bn_scale + bn_shift), written into the SPLIT o1p:
            # z0 = even orig w -> padded odd cols = split 23:45; z1 -> split 1:23
            for csl, z in ((slice(23, 45), z0), (slice(1, 23), z1)):
                nc.scalar.activation(
                    o1p_t[:, mch, 1 + r0:1 + r0 + WT, csl], z[:], AF.Relu,
                    bias=f32(prm_t[:, bnt_col + mch:bnt_col + mch + 1]),
                    scale=f32(prm_t[:, bns_col + mch:bns_col + mch + 1]))
            if nh == 0:
                # conv2-nh0's transform rows 0:23 depend only on this o1p half
                # (+borders): emit now so they run under conv1-nh1's matmuls,
                # leaving only a 1-row sliver at the conv1->conv2 boundary
                xt2 = ctmp.tile([128, 4, 24, WT], CONV_DT, tag="xt2", bufs=2,
                                name=f"xt2p_{mch}")
                Rp = slice(0, 23)
                nc.vector.tensor_tensor(xt2[:, 0, 0:23], o1p_t[:, mch, Rp, 0:22], o1p_t[:, mch, Rp, 1:23], ALU.subtract)
                nc.vector.tensor_tensor(xt2[:, 1, 0:23], o1p_t[:, mch, Rp, 23:45], o1p_t[:, mch, Rp, 1:23], ALU.add)
                nc.vector.tensor_tensor(xt2[:, 2, 0:23], o1p_t[:, mch, Rp, 1:23], o1p_t[:, mch, Rp, 23:45], ALU.subtract)
                nc.vector.tensor_tensor(xt2[:, 3, 0:23], o1p_t[:, mch, Rp, 23:45], o1p_t[:, mch, Rp, 24:46], ALU.subtract)
                c2pre[mch] = xt2

    if mid_hook is not None:
        mid_hook()
    # ---- conv2 Winograd: C=256 (2 ci) -> 2C=512, m in 2 passes of 2 chunks
    # (8 PSUM banks each); o1p is already in even/odd-split layout ----
    for nh in range(2):
        r0 = 22 * nh
        R = slice(r0, r0 + 24)
        xts, w2s = [], []
        for ci in range(2):
            w2c = wpool.tile([128, 12, 512], CONV_DT, tag="w", bufs=2,
                             name=f"w2g{nh}_{ci}")
            # split by m-half: the mp0 matmuls wait on only the first transfer
            nc.sync.dma_start(w2c[:, :, 0:256], w2_d[ci][:, :, 0:256])
            nc.sync.dma_start(w2c[:, :, 256:512], w2_d[ci][:, :, 256:512])
            w2s.append(w2c)
            if nh == 0 and ci in c2pre:
                xt2 = c2pre.pop(ci)
                Rs = slice(23, 24)  # row-23 sliver (needs conv1-nh1's output)
                nc.vector.tensor_tensor(xt2[:, 0, 23:24], o1p_t[:, ci, Rs, 0:22], o1p_t[:, ci, Rs, 1:23], ALU.subtract)
                nc.vector.tensor_tensor(xt2[:, 1, 23:24], o1p_t[:, ci, Rs, 23:45], o1p_t[:, ci, Rs, 1:23], ALU.add)
                nc.vector.tensor_tensor(xt2[:, 2, 23:24], o1p_t[:, ci, Rs, 1:23], o1p_t[:, ci, Rs, 23:45], ALU.subtract)
                nc.vector.tensor_tensor(xt2[:, 3, 23:24], o1p_t[:, ci, Rs, 23:45], o1p_t[:, ci, Rs, 24:46], ALU.subtract)
            else:
                xt2 = ctmp.tile([128, 4, 24, WT], CONV_DT, tag="xt2", bufs=2,
                                name=f"xt2_{nh}_{ci}")
                nc.vector.tensor_tensor(xt2[:, 0], o1p_t[:, ci, R, 0:22], o1p_t[:, ci, R, 1:23], ALU.subtract)
                nc.vector.tensor_tensor(xt2[:, 1], o1p_t[:, ci, R, 23:45], o1p_t[:, ci, R, 1:23], ALU.add)
                nc.vector.tensor_tensor(xt2[:, 2], o1p_t[:, ci, R, 1:23], o1p_t[:, ci, R, 23:45], ALU.subtract)
                nc.vector.tensor_tensor(xt2[:, 3], o1p_t[:, ci, R, 23:45], o1p_t[:, ci, R, 24:46], ALU.subtract)
            xts.append(xt2)
        zk = {}
        for mp in range(2):  # mp0 -> o2w chunks (m 0,1), mp1 -> o2b (m 2,3)
            ps2 = {}
            for mm_ in range(2):
                for wc in range(4):
                    ps2[(mm_, wc)] = cps.tile([128, WT * WT], F32, tag="cps",
                                              name=f"c2p_{nh}_{mp}_{mm_}_{wc}")
            for ci in range(2):
                for mm_ in range(2):
                    m = 2 * mp + mm_
                    for dy in range(3):
                        for wc in range(4):
                            _mm(nc, "conv2",
                                ps2[(mm_, wc)][:].rearrange("p (a b) -> p a b", a=WT),
                                w2s[ci][:, 4 * dy + wc, 128 * m:128 * (m + 1)],
                                xts[ci][:, wc, dy:dy + WT, :],
                                start=(ci == 0 and dy == 0),
                                stop=(ci == 1 and dy == 2),
                                skip_group_check=True)
            for mm_ in range(2):
                m0, m1, m2, m3 = (ps2[(mm_, wc)][:].rearrange("p (a b) -> p a b", a=WT)
                                  for wc in range(4))
                s = ctmp.tile([128, WT, WT], CONV_DT, tag="iv", bufs=6, name=f"s2_{nh}_{mp}_{mm_}")
                nc.scalar.copy(s[:], m1)
                a0 = ctmp.tile([128, WT, WT], CONV_DT, tag="iv", bufs=6, name=f"a2w_{nh}_{mp}_{mm_}")
                nc.vector.tensor_tensor(a0[:], m2, s[:], ALU.add)
                z0 = ctmp.tile([128, WT, WT], CONV_DT, tag="zk", bufs=8, name=f"z2e_{nh}_{mp}_{mm_}")
                nc.vector.tensor_tensor(z0[:], m0, a0[:], ALU.add)
                a1 = ctmp.tile([128, WT, WT], CONV_DT, tag="iv", bufs=6, name=f"a2b_{nh}_{mp}_{mm_}")
                nc.vector.tensor_tensor(a1[:], s[:], m2, ALU.subtract)
                z1 = ctmp.tile([128, WT, WT], CONV_DT, tag="zk", bufs=8, name=f"z2o_{nh}_{mp}_{mm_}")
                nc.vector.tensor_tensor(z1[:], a1[:], m3, ALU.subtract)
                zk[(mp, mm_, 0)] = z0
                zk[(mp, mm_, 1)] = z1
        # gating: out = relu((o2w + c2bw)*o1 + o2b + c2bb) per (mch, parity);
        # parity 0 = even orig w = split o1 cols 23:45, parity 1 = split 1:23
        for mch in range(2):
            for par in range(2):
                o1s = o1p_t[:, mch, 1 + r0:1 + r0 + WT,
                            slice(23, 45) if par == 0 else slice(1, 23)]
                t1 = ctmp.tile([128, WT, WT], F32, tag="g1", name=f"t1_{nh}_{mch}_{par}")
                nc.vector.scalar_tensor_tensor(
                    t1[:], zk[(0, mch, par)][:],
                    f32(prm_t[:, c2b_col + mch:c2b_col + mch + 1]),
                    o1s, ALU.add, ALU.mult)
                t2 = ctmp.tile([128, WT, WT], F32, tag="g2", name=f"t2_{nh}_{mch}_{par}")
                nc.vector.scalar_tensor_tensor(
                    t2[:], zk[(1, mch, par)][:],
                    f32(prm_t[:, c2b_col + mch + 2:c2b_col + mch + 3]),
                    t1[:], ALU.add, ALU.add)
                ov = out_t[:, mch, W * r0:W * (r0 + WT)].rearrange(
                    "p (r j t) -> p r j t", r=WT, t=2)
                nc.scalar.activation(ov[:, :, :, par], t2[:], AF.Relu)


def _att_weights(nc, qkw_d, vw_d, pool, tags):
    qkw_t = pool.tile([128, 2, 64], F32R, tag=tags + "qkw", name=tags + "qkw")
    vw_t = pool.tile([128, 2, 256], F32R, tag=tags + "vw", name=tags + "vw")
    for kc in range(2):
        nc.sync.dma_start(qkw_t[:, kc, :], qkw_d[kc].bitcast(F32R))
        nc.sync.dma_start(vw_t[:, kc, :], vw_d[kc].bitcast(F32R))
    return qkw_t, vw_t


def _att_qk_alloc(nc, prm_t, pool, tags):
    """Allocate q4 [128, N] / k4 [128, NK]: 4 replicated 32-row bands for the
    row-tiled score matmuls. k4 band-0 cols 1936:NK are zeroed (phantom n for
    jn=15: scores 0 -> exp 1 -> killed by vT zero rows; z uses the partial-ones
    col); replication DMAs propagate the zeros to bands 1-3."""
    zw = prm_t[:, C_ZEROW:C_ZEROW + PX]
    q4_t = pool.tile([128, N], BF16, tag=tags + "q", name=tags + "q")
    k4_t = pool.tile([128, NK], BF16, tag=tags + "k", name=tags + "k")
    nc.vector.tensor_copy(k4_t[0:32, N:NK], zw[0:32, 0:NK - N])
    return q4_t, k4_t


def _att_qk(nc, qkw_t, qb_col, kb_col, src_qk, prm_t, q4_t, k4_t, pspool, tags,
            ps_tag="cps", ps_bufs=None):
    """Fill q4/k4 band 0 from src_qk projections, then replicate bands via
    SBUF->SBUF DMA (no compute-engine cost; consumers are far downstream)."""
    f32 = lambda ap: ap.bitcast(F32)
    kw = dict(bufs=ps_bufs) if ps_bufs is not None else {}
    for im in range(NCH):
        msl = slice(PX * im, PX * (im + 1))
        pq = pspool.tile([64, PX], F32, tag=ps_tag, name=tags + f"pq{im}", **kw)
        for kc in range(2):
            _mm(nc, 'qk', pq[:], qkw_t[:, kc, :], src_qk[:, kc, msl],
                start=(kc == 0), stop=(kc == 1), skip_group_check=True)
        nc.vector.tensor_scalar_add(q4_t[0:32, msl], pq[0:32, :], f32(prm_t[0:32, qb_col:qb_col + 1]))
        nc.vector.tensor_scalar_add(k4_t[0:32, msl], pq[32:64, :], f32(prm_t[0:32, kb_col:kb_col + 1]))
    for t, w in ((q4_t, N), (k4_t, NK)):
        for b in range(1, 4):
            nc.sync.dma_start(t[32 * b:32 * b + 32, 0:w], t[0:32, 0:w])


def _att_v_alloc(nc, prm_t, pool, tags):
    zw = prm_t[:, C_ZEROW:C_ZEROW + PX]
    vT_t = pool.tile([128, AJ, 256], F32R, tag=tags + "vT", name=tags + "vT")
    nc.vector.tensor_copy(vT_t[:, AJ - 1, :], zw[:, 0:256])
    return vT_t


def _att_v_pairs(nc, vw_t, src_v, vT_t, pspool, tags, jps,
                 ps_tag="cps", ps_bufs=None, dve_only=False):
    """vT chunk pairs (2 jn per PSUM bank); rows 16:128 of the last chunk stay
    zeroed. Drains alternate scalar/vector unless dve_only (keeps ACT free)."""
    kw = dict(bufs=ps_bufs) if ps_bufs is not None else {}
    for jp in jps:
        tg = ps_tag[jp % len(ps_tag)] if isinstance(ps_tag, (list, tuple)) else ps_tag
        pv = pspool.tile([128, 2, 256], F32, tag=tg, name=tags + f"pv{jp}", **kw)
        for h in range(2):
            jn = 2 * jp + h
            nsz = 128 if jn < AJ - 1 else 16
            for kc in range(2):
                _mm(nc, 'vT', pv[0:nsz, h, :],
                    src_v[:, kc, 128 * jn:128 * jn + nsz],
                    vw_t[:, kc, :],
                    start=(kc == 0), stop=(kc == 1), skip_group_check=True)
        if jp < AJ // 2 - 1:
            if jp % 2 == 0 and not dve_only:
                nc.scalar.copy(vT_t[:, 2 * jp:2 * jp + 2, :], pv[:, :, :])
            else:
                nc.vector.tensor_copy(vT_t[:, 2 * jp:2 * jp + 2, :], pv[:, :, :])
        else:
            if dve_only:
                nc.vector.tensor_copy(vT_t[:, AJ - 2, :], pv[:, 0, :])
            else:
                nc.scalar.copy(vT_t[:, AJ - 2, :], pv[:, 0, :])
            nc.vector.tensor_copy(vT_t[0:16, AJ - 1, :], pv[0:16, 1, :])


def _att_pipeline(nc, atts, prm_t, aps, atmp, epool, prologue):
    """Pipelined attention over 8 chunks (2 atts x 4 m-blocks of 484).

    Per chunk: 4 ladder passes, each = 4 concurrently-executing row-tiled K=32
    score matmuls (tile_position bands) into 4 PSUM banks, drained by ONE wide
    exp into eT. Then 16 z + 32 feat matmuls (full array), normalize via
    approx-reciprocal + gpsimd broadcast, residual add, DMA out. Chunk c+1's
    ladder passes are interleaved with chunk c's z/feat groups so the PE never
    sits behind ACT exp; `prologue` callables fill chunk 0's exp-wait gaps.
    """
    f32 = lambda ap: ap.bitcast(F32)
    eTs, state = {}, {}

    def emit_z(c, jns):
        att, eT = atts[c // 4], eTs[c]
        for jn in jns:
            oc = C_ONESP if jn == AJ - 1 else C_ONESC
            _mm(nc, 'z', state[c]['pz'][0:1, :], prm_t[:, oc:oc + 1], eT[:, jn, :],
                start=(jn == 0), stop=(jn == AJ - 1), skip_group_check=True)

    def ladder_pass(c, p):
        att, im = atts[c // 4], c % 4
        msl = slice(PX * im, PX * (im + 1))
        if p == 0:
            eTs[c] = epool.tile([128, AJ, PX], F32R, tag="eT", name=f"eT{c}")
            state[c] = dict(
                pf=aps.tile([128, 2, 512], F32, tag="f", bufs=1, name=f"pf_{c}"),
                pz=aps.tile([1, PX], F32, tag="z", bufs=1, name=f"pz_{c}"))
        pst = aps.tile([128, 4, 512], F32, tag="st", bufs=1, name=f"pst_{c}_{p}")
        for r in range(4):
            jn = 4 * p + r
            _mm(nc, 'sT', pst[0:128, r, 0:PX],
                att['k4'][32 * r:32 * r + 32, 128 * jn:128 * jn + 128],
                att['q4'][32 * r:32 * r + 32, msl],
                start=True, stop=True, skip_group_check=True,
                tile_position=(32 * r, 0))
        nc.scalar.activation(eTs[c][:, 4 * p:4 * p + 4, :], pst[:, :, 0:PX], AF.Exp)
        if p >= 1:
            # z for the previous pass's (already exp'd) chunks: pz completes at
            # the start of chunk c's compute, so 1/Z + broadcast run a full
            # chunk ahead of the fo muls that consume them
            emit_z(c, range(4 * (p - 1), 4 * p))

    def group(c, g):
        att = atts[c // 4]
        eT = eTs[c]
        st = state[c]
        if g == 0:
            emit_z(c, range(AJ - 4, AJ))
            invz = atmp.tile([1, PX], F32, tag="invz", bufs=2, name=f"invz{c}")
            nc.vector.reciprocal_approx_fast(invz[0:1, :], st['pz'][0:1, :])
            izb = atmp.tile([128, PX], F32, tag="izb", bufs=2, name=f"izb{c}")
            nc.gpsimd.partition_broadcast(izb[:], invz[0:1, :])
            st['izb'] = izb
        if c == 7 and g == 3:
            # last chunk's last group: split feats by column half (first half
            # finishes early so its normalize/DMA chain hides under the second)
            for a, b in ((0, PX // 2), (PX // 2, PX)):
                for r in range(4):
                    jn = 4 * g + r
                    for h in range(2):
                        _mm(nc, 'feat', st['pf'][:, h, a:b],
                            att['vT'][:, jn, 128 * h:128 * h + 128], eT[:, jn, a:b],
                            start=False, stop=(jn == AJ - 1), skip_group_check=True)
        else:
            for r in range(4):
                jn = 4 * g + r
                for h in range(2):
                    _mm(nc, 'feat', st['pf'][:, h, 0:PX],
                        att['vT'][:, jn, 128 * h:128 * h + 128], eT[:, jn, :],
                        start=(jn == 0), stop=(jn == AJ - 1), skip_group_check=True)

    def post(c):
        att, im = atts[c // 4], c % 4
        st = state.pop(c)
        # last chunk: split into halves so DVE/DMA pipeline on the exposed tail
        spans = ((0, PX // 2), (PX // 2, PX)) if c == 7 else ((0, PX),)
        for cch in range(2):
            for a, b in spans:
                csl = slice(PX * im + a, PX * im + b)
                fo = atmp.tile([128, b - a], F32, tag="fo", bufs=4, name=f"fo{c}_{cch}{a}")
                nc.vector.tensor_mul(fo[:], st['pf'][:, cch, a:b], st['izb'][:, a:b])
                oo = atmp.tile([128, b - a], F32, tag="oo", bufs=4, name=f"oo{c}_{cch}{a}")
                # out = (feat/Z + vb) + r   (v-bias folded here: sum(mask)=1)
                nc.vector.scalar_tensor_tensor(
                    oo[:], fo[:], f32(prm_t[:, att['vb'] + cch:att['vb'] + cch + 1]),
                    f32(att['res'][:, cch, csl]), ALU.add, ALU.add)
                nc.sync.dma_start(att['out'][cch, :, csl], oo[:])
        eTs.pop(c)

    for p in range(4):
        ladder_pass(0, p)
        if p < len(prologue):
            prologue[p]()
    for c in range(8):
        for g in range(4):
            if c < 7:
                ladder_pass(c + 1, g)
            group(c, g)
        post(c)


def build_nc():
    nc = bacc.Bacc(None)
    d = {}
    cdt = CONV_DT if BF16_CONV else F32
    d['xr'] = nc.dram_tensor("xr", [4, 128, HP, WP], cdt, kind="ExternalInput")
    d['xd'] = nc.dram_tensor("xd", [4, 128, HP, WP], cdt, kind="ExternalInput")
    d['w1r'] = nc.dram_tensor("w1r", [4, 128, 12, 256], cdt, kind="ExternalInput")
    d['w2r'] = nc.dram_tensor("w2r", [2, 128, 12, 512], cdt, kind="ExternalInput")
    d['w1d'] = nc.dram_tensor("w1d", [4, 128, 12, 256], cdt, kind="ExternalInput")
    d['w2d'] = nc.dram_tensor("w2d", [2, 128, 12, 512], cdt, kind="ExternalInput")
    for a in (1, 2):
        d[f'qkw{a}'] = nc.dram_tensor(f"qkw{a}", [2, 128, 64], F32, kind="ExternalInput")
        d[f'vw{a}'] = nc.dram_tensor(f"vw{a}", [2, 128, 256], F32, kind="ExternalInput")
    d['prm'] = nc.dram_tensor("prm", [128, PRM_COLS], F32, kind="ExternalInput")
    d['o1'] = nc.dram_tensor("o1", [2, 128, N], F32, kind="ExternalOutput")
    d['o2'] = nc.dram_tensor("o2", [2, 128, N], F32, kind="ExternalOutput")

    with tile.TileContext(nc) as tc:
        with tc.tile_pool(name="persist", bufs=1) as persist, \
             tc.tile_pool(name="aearly", bufs=1) as aearly:
            prm_t = persist.tile([128, PRM_COLS], F32R, tag="prm")
            nc.sync.dma_start(prm_t[:], d['prm'][:].bitcast(F32R))
            r_t = persist.tile([128, 2, N], F32R, tag="r")
            d_t = persist.tile([128, 2, N], F32R, tag="d")

            with tc.tile_pool(name="wpool", bufs=3) as wpool, \
                 tc.tile_pool(name="xpool", bufs=3) as xpool, \
                 tc.tile_pool(name="o1pool", bufs=1) as o1pool, \
                 tc.tile_pool(name="cps", bufs=8, space="PSUM") as cps, \
                 tc.tile_pool(name="ctmp", bufs=3) as ctmp:
                o1p_t = o1pool.tile([128, 2, HP, WP], CONV_DT, tag="o1p")
                dpre = {}

                def _depth_prefetch():
                    # depth stream's (nh0, ci0) x + weight + transform, emitted
                    # between rgb's conv1 and conv2 so the transform runs under
                    # conv2_r instead of behind its combine backlog
                    xpc = xpool.tile([128, HP, WP], CONV_DT, tag="xpad", bufs=4,
                                     name="xpad_d0")
                    nc.sync.dma_start(xpc[:, 0:24, :], d['xd'][0][:, 0:24, :])
                    nc.sync.dma_start(xpc[:, 24:HP, :], d['xd'][0][:, 24:HP, :])
                    w1c = wpool.tile([128, 12, 256], CONV_DT, tag="w1g", bufs=3,
                                     name="w1g_d0")
                    nc.sync.dma_start(w1c[:, 0:4, :], d['w1d'][0][:, 0:4, :])
                    nc.sync.dma_start(w1c[:, 4:12, :], d['w1d'][0][:, 4:12, :])
                    xt = ctmp.tile([128, 4, 24, 22], CONV_DT, tag="xt", bufs=3,
                                   name="xt_d0")
                    R = slice(0, 24)
                    nc.vector.tensor_tensor(xt[:, 0], xpc[:, R, 0:22], xpc[:, R, 1:23], ALU.subtract)
                    nc.vector.tensor_tensor(xt[:, 1], xpc[:, R, 23:45], xpc[:, R, 1:23], ALU.add)
                    nc.vector.tensor_tensor(xt[:, 2], xpc[:, R, 1:23], xpc[:, R, 23:45], ALU.subtract)
                    nc.vector.tensor_tensor(xt[:, 3], xpc[:, R, 23:45], xpc[:, R, 24:46], ALU.subtract)
                    dpre.update(xpc=xpc, w1c=w1c, xt=xt)

                _conv_stream(nc, tc, d['xr'], d['w1r'], d['w2r'],
                             C_BNS1, C_BNT1, C_C2B1, prm_t, o1p_t, r_t,
                             wpool, xpool, cps, ctmp, True,
                             mid_hook=_depth_prefetch)
                # rgb-dependent attention preps run while depth convs stream:
                # att1 v comes from r, att2 q/k come from r
                qkw1_t, vw1_t = _att_weights(nc, d['qkw1'], d['vw1'], aearly, "a1")
                qkw2_t, vw2_t = _att_weights(nc, d['qkw2'], d['vw2'], aearly, "a2")
                q1_t, k1_t = _att_qk_alloc(nc, prm_t, aearly, "a1")
                q2_t, k2_t = _att_qk_alloc(nc, prm_t, aearly, "a2")
                vT1_t = _att_v_alloc(nc, prm_t, aearly, "a1")
                vT2_t = _att_v_alloc(nc, prm_t, aearly, "a2")
                _att_qk(nc, qkw2_t, C_QB2, C_KB2, r_t, prm_t, q2_t, k2_t, cps, "a2")
                _conv_stream(nc, tc, d['xd'], d['w1d'], d['w2d'],
                             C_BNS2, C_BNT2, C_C2B2, prm_t, o1p_t, d_t,
                             wpool, xpool, cps, ctmp, False, pre_x=dpre)
                # vT1 last: its consumer (att1, chunks 4-7) is ~170us away, its
                # drains queued mid-stream were stalling the PE between convs,
                # and its matmuls now fill the psum-pool close barrier
                _att_v_pairs(nc, vw1_t, r_t, vT1_t, cps, "a1", range(AJ // 2))

            with tc.tile_pool(name="aps", bufs=2, space="PSUM") as aps, \
                 tc.tile_pool(name="atmp", bufs=2) as atmp, \
                 tc.tile_pool(name="epool", bufs=2) as epool:
                # att2 first: its q/k come from r_t (ready since mid-depth-conv), so
                # its score ladder starts immediately after the depth conv; att1's
                # q/k projections + vT2 fill chunk-0's exp-wait gaps (prologue),
                # and att1's band fills complete while att2 runs.
                atts = [
                    dict(q4=q2_t, k4=k2_t, vT=vT2_t, vb=C_VB2, res=d_t, out=d['o2']),
                    dict(q4=q1_t, k4=k1_t, vT=vT1_t, vb=C_VB1, res=r_t, out=d['o1']),
                ]
                prologue = [
                    lambda: _att_qk(nc, qkw1_t, C_QB1, C_KB1, d_t, prm_t,
                                    q1_t, k1_t, aps, "a1", ps_tag="pq", ps_bufs=1),
                    lambda: _att_v_pairs(nc, vw2_t, d_t, vT2_t, aps, "a2",
                                         range(0, AJ // 4), ps_tag=("f", "pq"),
                                         ps_bufs=1, dve_only=True),
                    lambda: _att_v_pairs(nc, vw2_t, d_t, vT2_t, aps, "a2",
                                         range(AJ // 4, AJ // 2), ps_tag=("f", "pq"),
                                         ps_bufs=1, dve_only=True),
                ]
                _att_pipeline(nc, atts, prm_t, aps, atmp, epool, prologue)

    nc.finalize()
    return nc


def _prep_common(g):
    """Host-side weight layout prep (shared across cores)."""
    out = {}
    for pre, kw1, kw2 in (('sa1', 'w1r', 'w2r'), ('sa2', 'w1d', 'w2d')):
        c1w = g[f'{pre}_c1_w']  # [256, 512, 3, 3]
        c2w = g[f'{pre}_c2_w']  # [512, 256, 3, 3]
        cnp = ml_dtypes.bfloat16 if BF16_CONV else np.float32
        # conv1: 1D-Winograd G-transform along dx: 3 taps -> 4 w-coords
        w = c1w.transpose(1, 2, 3, 0)  # [512, 3dy, 3dx, 256]
        wg = np.stack([w[:, :, 0], (w[:, :, 0] + w[:, :, 1] + w[:, :, 2]) * 0.5,
                       (w[:, :, 0] - w[:, :, 1] + w[:, :, 2]) * 0.5, w[:, :, 2]],
                      axis=2)  # [512, 3dy, 4wc, 256]
        out[kw1] = np.ascontiguousarray(wg.reshape(4, 128, 12, 256).astype(cnp))
        w2 = c2w.transpose(1, 2, 3, 0)  # [256, 3dy, 3dx, 512]
        wg2 = np.stack([w2[:, :, 0], (w2[:, :, 0] + w2[:, :, 1] + w2[:, :, 2]) * 0.5,
                        (w2[:, :, 0] - w2[:, :, 1] + w2[:, :, 2]) * 0.5, w2[:, :, 2]],
                       axis=2)  # [256, 3dy, 4wc, 512]
        out[kw2] = np.ascontiguousarray(wg2.reshape(2, 128, 12, 512).astype(cnp))

    gate = float(g['gate'][0]); beta = float(g['beta'][0]); gamma = float(g['gamma'][0])
    s1 = gate * beta
    s2 = (1.0 - gate) * gamma
    for a, s in ((1, s1), (2, s2)):
        vw = (s * g[f'a{a}_vw']).astype(np.float32)
        qkw = np.concatenate([g[f'a{a}_qw'], g[f'a{a}_kw']], axis=0)  # [64, 256]
        out[f'qkw{a}'] = np.ascontiguousarray(qkw.T.reshape(2, 128, 64))
        out[f'vw{a}'] = np.ascontiguousarray(vw.T.reshape(2, 128, 256))

    prm = np.zeros((128, PRM_COLS), np.float32)
    for pre, cs, ct, cb in (('sa1', C_BNS1, C_BNT1, C_C2B1), ('sa2', C_BNS2, C_BNT2, C_C2B2)):
        s = (g[f'{pre}_bn_g'] / np.sqrt(g[f'{pre}_bn_v'] + EPS)).astype(np.float32)
        t = ((g[f'{pre}_c1_b'] - g[f'{pre}_bn_m']) * s + g[f'{pre}_bn_b']).astype(np.float32)
        prm[:, cs:cs + 2] = s.reshape(2, 128).T
        prm[:, ct:ct + 2] = t.reshape(2, 128).T
        prm[:, cb:cb + 4] = g[f'{pre}_c2_b'].reshape(4, 128).T
    prm[0:32, C_QB1] = g['a1_qb']; prm[0:32, C_KB1] = g['a1_kb']
    prm[0:32, C_QB2] = g['a2_qb']; prm[0:32, C_KB2] = g['a2_kb']
    prm[:, C_VB1:C_VB1 + 2] = (s1 * g['a1_vb']).astype(np.float32).reshape(2, 128).T
    prm[:, C_VB2:C_VB2 + 2] = (s2 * g['a2_vb']).astype(np.float32).reshape(2, 128).T
    prm[0, C_ONESR:C_ONESR + 128] = 1.0
    prm[:, C_ONESC] = 1.0
    prm[0:16, C_ONESP] = 1.0   # z station for jn=15: ignore phantom rows 16:128
    out['prm'] = prm
    return out


def _prep_x(x):
    """[512, 44, 44] -> padded even/odd-split [4, 128, 46, 46]:
    cols 0:23 = padded even cols, 23:46 = padded odd cols."""
    p = np.zeros((512, HP, WP), np.float32)
    p[:, 1:45, 1:45] = x
    ps = np.concatenate([p[:, :, 0::2], p[:, :, 1::2]], axis=2)
    return ps.reshape(4, 128, HP, WP).astype(
        ml_dtypes.bfloat16 if BF16_CONV else np.float32)


_NC_CACHE = None


def kernel(**inputs):
    global _NC_CACHE
    g = {k: np.asarray(v, np.float32) for k, v in inputs.items()}
    if _NC_CACHE is None:
        _NC_CACHE = build_nc()
    nc = _NC_CACHE

    common = _prep_common(g)
    B = g['rgb'].shape[0]
    in_maps = []
    for b in range(B):
        m = dict(common)
        m['xr'] = _prep_x(g['rgb'][b])
        m['xd'] = _prep_x(g['depth'][b])
        in_maps.append(m)

    res = run_bass_kernel_spmd(nc, in_maps, list(range(B)))
    out1 = np.stack([res.results[b]['o1'].reshape(256, H, W) for b in range(B)])
    out2 = np.stack([res.results[b]['o2'].reshape(256, H, W) for b in range(B)])
    return out1, out2



# revision 69
# speedup vs baseline: 1.0040x; 1.0040x over previous
"""TRN2 Bass kernel for nn_CMAT_4561255269047 (dual-stream CNN + cross-attention).

Data-parallel over batch B=8 across 8 NeuronCores (1 sample/core, no collectives).

Per-core program (all matmuls fp32r at full PE rate):
  conv3x3 = 9 shifted matmuls over zero-padded [C,46,46] images, accumulated in
  PSUM over input-channel chunks (ci-outer loop, 8 PSUM banks resident).
  conv1 -> BN+ReLU fused into the PSUM-drain activation (scale/bias APs).
  conv2 -> gated residual relu((o2w+b)*o1 + (o2b+b)) via scalar_tensor_tensor.
  attention: sT[n,m] = k^T q (K=32), eT = exp(sT) (scores are small, no max
  subtraction), feat[c,m] = vT^T @ eT, Z[m] via ones-column matmul, normalize
  by 1/Z broadcast through a K=1 matmul, residual add, DMA out per chunk.
  gate*beta / (1-gate)*gamma are folded into vw/vb on the host.
"""
import sys
sys.path.insert(0, '/opt/trn_rl_repo')

import numpy as np
import ml_dtypes

import concourse.bass as bass
import concourse.mybir as mybir
import concourse.tile as tile
from concourse import bacc
from concourse.bass_utils import run_bass_kernel_spmd

MM_KINDS = {}

F32 = mybir.dt.float32
F32R = mybir.dt.float32r
BF16 = mybir.dt.bfloat16
BF16_CONV = True  # bf16 convs save ~27us but cost 12x accuracy (6e-3 vs 5e-4)
CONV_DT = BF16 if BF16_CONV else F32R
EPS = 1e-5
AF = mybir.ActivationFunctionType
ALU = mybir.AluOpType

H = W = 44
HP = WP = 46
N = H * W            # 1936
NCH = 4              # spatial n-chunks of 11 rows (484 px) for convs / att m
ROWS = 11
PX = ROWS * W        # 484
AJ = 16              # attention n-chunks of 128 (last = 16)

# prm packed-param columns
C_BNS1, C_BNT1, C_BNS2, C_BNT2 = 0, 2, 4, 6
C_C2B1, C_C2B2 = 8, 12
C_QB1, C_KB1, C_QB2, C_KB2 = 16, 17, 18, 19
C_VB1, C_VB2 = 20, 22            # v-bias as per-partition scalars, 2 c-chunks each
C_ONESR, C_ONESC = 24, 152       # ones row (partition 0) / ones column
C_ZERO = 153                     # 46 zero cols (o1p border source)
C_ONESP = 199                    # partial ones column (rows 0:16) for the last z chunk
C_ZEROW = 200                    # 484 zero cols (K-padding source)
PRM_COLS = 684
NK = 2048                        # k4 columns: N padded so jn=15 runs full M=128


def _mm(nc, kind, *args, **kw):
    inst = nc.tensor.matmul(*args, **kw)
    try:
        MM_KINDS[inst.ins.name] = kind
    except Exception:
        pass
    return inst


def _conv_stream(nc, tc, x_d, w1_d, w2_d, bns_col, bnt_col, c2b_col,
                 prm_t, o1p_t, out_t, wpool, xpool, cps, ctmp, zero_borders,
                 mid_hook=None):
    """One sa_block: conv1 (1D-Winograd F(2,3) along W) -> BN+relu -> o1p_t
    (padded, normal layout via stride-2 stores), conv2 + gating -> out_t.

    x_d arrives in even/odd-split layout: cols 0:23 = even padded cols,
    23:46 = odd. w1_d is G-transformed on the host: [ci, 128, 3dy*4wc, 256].
    """
    f32 = lambda ap: ap.bitcast(F32)

    if zero_borders:
        # zero the o1p padding ring once (interior is fully overwritten per stream)
        zsrc = prm_t[:, C_ZERO:C_ZERO + HP]
        for ci in range(2):
            nc.vector.tensor_copy(o1p_t[:, ci, 0, :], zsrc)
            nc.vector.tensor_copy(o1p_t[:, ci, HP - 1, :], zsrc)
            nc.vector.tensor_copy(o1p_t[:, ci, :, 0], zsrc)
            nc.vector.tensor_copy(o1p_t[:, ci, :, HP - 1], zsrc)

    # ---- conv1 Winograd: Cin=512 (4 resident ci chunks) -> C=256 (2 m chunks),
    # rows in 2 halves of 22, w in 22 tiles of 2 outputs ----
    xs = []
    WT = 22  # w tiles / output cols per parity

    def _wdma_transform(nh, ci):
        w1c = wpool.tile([128, 12, 256], CONV_DT, tag="w1g", bufs=3,
                         name=f"w1g{nh}_{ci}")
        # dy=0 taps first: the first matmuls wait on only 1/3 of the transfer
        nc.sync.dma_start(w1c[:, 0:4, :], w1_d[ci][:, 0:4, :])
        nc.sync.dma_start(w1c[:, 4:12, :], w1_d[ci][:, 4:12, :])
        R = slice(22 * nh, 22 * nh + 24)
        xpc = xs[ci]
        xt = ctmp.tile([128, 4, 24, WT], CONV_DT, tag="xt", bufs=3, name=f"xt{nh}_{ci}")
        # B^T d per w-tile: d = [ev_j, od_j, ev_j1, od_j1]
        nc.vector.tensor_tensor(xt[:, 0], xpc[:, R, 0:22], xpc[:, R, 1:23], ALU.subtract)
        nc.vector.tensor_tensor(xt[:, 1], xpc[:, R, 23:45], xpc[:, R, 1:23], ALU.add)
        nc.vector.tensor_tensor(xt[:, 2], xpc[:, R, 1:23], xpc[:, R, 23:45], ALU.subtract)
        nc.vector.tensor_tensor(xt[:, 3], xpc[:, R, 23:45], xpc[:, R, 24:46], ALU.subtract)
        return w1c, xt

    pre = {}
    c2pre = {}
    for ci in range(4):
        xpc = xpool.tile([128, HP, WP], CONV_DT, tag="xpad", bufs=4, name=f"xpad{ci}")
        nc.sync.dma_start(xpc[:, 0:24, :], x_d[ci][:, 0:24, :])
        xs.append(xpc)
        if ci == 0:
            # weight DMA + transform right behind ci0's FIRST x half (the nh0
            # transform reads only rows 0:24), so the first matmul's critical
            # DMA chain is just prm + x-half + w[0:4]; rows 24:46 follow after
            pre[(0, 0)] = _wdma_transform(0, 0)
        nc.sync.dma_start(xpc[:, 24:HP, :], x_d[ci][:, 24:HP, :])
    for nh in range(2):
        r0 = 22 * nh
        ps = {}
        for mch in range(2):
            for wc in range(4):
                ps[(mch, wc)] = cps.tile([128, WT * WT], F32, tag="cps",
                                         name=f"c1p_{nh}_{mch}_{wc}")
        for ci in range(4):
            w1c, xt = pre.pop((nh, ci), None) or _wdma_transform(nh, ci)
            for mch in range(2):
                for dy in range(3):
                    for wc in range(4):
                        _mm(nc, "conv1",
                            ps[(mch, wc)][:].rearrange("p (a b) -> p a b", a=WT),
                            w1c[:, 4 * dy + wc, 128 * mch:128 * (mch + 1)],
                            xt[:, wc, dy:dy + WT, :],
                            start=(ci == 0 and dy == 0),
                            stop=(ci == 3 and dy == 2),
                            skip_group_check=True)
        if nh == 0:
            # prefetch next half's transforms ahead of the inverse ops in the
            # DVE queue so the nh=1 matmuls aren't blocked behind them
            pre[(1, 0)] = _wdma_transform(1, 0)
            pre[(1, 1)] = _wdma_transform(1, 1)
        for mch in range(2):
            m0, m1, m2, m3 = (ps[(mch, wc)][:].rearrange("p (a b) -> p a b", a=WT)
                              for wc in range(4))
            s = ctmp.tile([128, WT, WT], CONV_DT, tag="iv", bufs=6, name=f"s{nh}_{mch}")
            nc.scalar.copy(s[:], m1)
            a0 = ctmp.tile([128, WT, WT], CONV_DT, tag="iv", bufs=6, name=f"a0{nh}_{mch}")
            nc.vector.tensor_tensor(a0[:], m2, s[:], ALU.add)
            z0 = ctmp.tile([128, WT, WT], CONV_DT, tag="iv", bufs=6, name=f"z0{nh}_{mch}")
            nc.vector.tensor_tensor(z0[:], m0, a0[:], ALU.add)
            a1 = ctmp.tile([128, WT, WT], CONV_DT, tag="iv", bufs=6, name=f"a1{nh}_{mch}")
            nc.vector.tensor_tensor(a1[:], s[:], m2, ALU.subtract)
            z1 = ctmp.tile([128, WT, WT], CONV_DT, tag="iv", bufs=6, name=f"z1{nh}_{mch}")
            nc.vector.tensor_tensor(z1[:], a1[:], m3, ALU.subtract)
            # o1 = relu(z * bn_scale + bn_shift), written into the SPLIT o1p:
            # z0 = even orig w -> padded odd cols = split 23:45; z1 -> split 1:23
            for csl, z in ((slice(23, 45), z0), (slice(1, 23), z1)):
                nc.scalar.activation(
                    o1p_t[:, mch, 1 + r0:1 + r0 + WT, csl], z[:], AF.Relu,
                    bias=f32(prm_t[:, bnt_col + mch:bnt_col + mch + 1]),
                    scale=f32(prm_t[:, bns_col + mch:bns_col + mch + 1]))
            if nh == 0:
                # conv2-nh0's transform rows 0:23 depend only on this o1p half
                # (+borders): emit now so they run under conv1-nh1's matmuls,
                # leaving only a 1-row sliver at the conv1->conv2 boundary
                xt2 = ctmp.tile([128, 4, 24, WT], CONV_DT, tag="xt2", bufs=2,
                                name=f"xt2p_{mch}")
                Rp = slice(0, 23)
                nc.vector.tensor_tensor(xt2[:, 0, 0:23], o1p_t[:, mch, Rp, 0:22], o1p_t[:, mch, Rp, 1:23], ALU.subtract)
                nc.vector.tensor_tensor(xt2[:, 1, 0:23], o1p_t[:, mch, Rp, 23:45], o1p_t[:, mch, Rp, 1:23], ALU.add)
                nc.vector.tensor_tensor(xt2[:, 2, 0:23], o1p_t[:, mch, Rp, 1:23], o1p_t[:, mch, Rp, 23:45], ALU.subtract)
                nc.vector.tensor_tensor(xt2[:, 3, 0:23], o1p_t[:, mch, Rp, 23:45], o1p_t[:, mch, Rp, 24:46], ALU.subtract)
                c2pre[mch] = xt2

    if mid_hook is not None:
        mid_hook()
    # ---- conv2 Winograd: C=256 (2 ci) -> 2C=512, m in 2 passes of 2 chunks
    # (8 PSUM banks each); o1p is already in even/odd-split layout ----
    for nh in range(2):
        r0 = 22 * nh
        R = slice(r0, r0 + 24)
        xts, w2s = [], []
        for ci in range(2):
            w2c = wpool.tile([128, 12, 512], CONV_DT, tag="w", bufs=2,
                             name=f"w2g{nh}_{ci}")
            # split by m-half: the mp0 matmuls wait on only the first transfer
            nc.sync.dma_start(w2c[:, :, 0:256], w2_d[ci][:, :, 0:256])
            nc.sync.dma_start(w2c[:, :, 256:512], w2_d[ci][:, :, 256:512])
            w2s.append(w2c)
            if nh == 0 and ci in c2pre:
                xt2 = c2pre.pop(ci)
                Rs = slice(23, 24)  # row-23 sliver (needs conv1-nh1's output)
                nc.vector.tensor_tensor(xt2[:, 0, 23:24], o1p_t[:, ci, Rs, 0:22], o1p_t[:, ci, Rs, 1:23], ALU.subtract)
                nc.vector.tensor_tensor(xt2[:, 1, 23:24], o1p_t[:, ci, Rs, 23:45], o1p_t[:, ci, Rs, 1:23], ALU.add)
                nc.vector.tensor_tensor(xt2[:, 2, 23:24], o1p_t[:, ci, Rs, 1:23], o1p_t[:, ci, Rs, 23:45], ALU.subtract)
                nc.vector.tensor_tensor(xt2[:, 3, 23:24], o1p_t[:, ci, Rs, 23:45], o1p_t[:, ci, Rs, 24:46], ALU.subtract)
            else:
                xt2 = ctmp.tile([128, 4, 24, WT], CONV_DT, tag="xt2", bufs=2,
                                name=f"xt2_{nh}_{ci}")
                nc.vector.tensor_tensor(xt2[:, 0], o1p_t[:, ci, R, 0:22], o1p_t[:, ci, R, 1:23], ALU.subtract)
                nc.vector.tensor_tensor(xt2[:, 1], o1p_t[:, ci, R, 23:45], o1p_t[:, ci, R, 1:23], ALU.add)
                nc.vector.tensor_tensor(xt2[:, 2], o1p_t[:, ci, R, 1:23], o1p_t[:, ci, R, 23:45], ALU.subtract)
                nc.vector.tensor_tensor(xt2[:, 3], o1p_t[:, ci, R, 23:45], o1p_t[:, ci, R, 24:46], ALU.subtract)
            xts.append(xt2)
        zk = {}
        for mp in range(2):  # mp0 -> o2w chunks (m 0,1), mp1 -> o2b (m 2,3)
            ps2 = {}
            for mm_ in range(2):
                for wc in range(4):
                    ps2[(mm_, wc)] = cps.tile([128, WT * WT], F32, tag="cps",
                                              name=f"c2p_{nh}_{mp}_{mm_}_{wc}")
            for ci in range(2):
                for mm_ in range(2):
                    m = 2 * mp + mm_
                    for dy in range(3):
                        for wc in range(4):
                            _mm(nc, "conv2",
                                ps2[(mm_, wc)][:].rearrange("p (a b) -> p a b", a=WT),
                                w2s[ci][:, 4 * dy + wc, 128 * m:128 * (m + 1)],
                                xts[ci][:, wc, dy:dy + WT, :],
                                start=(ci == 0 and dy == 0),
                                stop=(ci == 1 and dy == 2),
                                skip_group_check=True)
            for mm_ in range(2):
                m0, m1, m2, m3 = (ps2[(mm_, wc)][:].rearrange("p (a b) -> p a b", a=WT)
                                  for wc in range(4))
                s = ctmp.tile([128, WT, WT], CONV_DT, tag="iv", bufs=6, name=f"s2_{nh}_{mp}_{mm_}")
                nc.scalar.copy(s[:], m1)
                a0 = ctmp.tile([128, WT, WT], CONV_DT, tag="iv", bufs=6, name=f"a2w_{nh}_{mp}_{mm_}")
                nc.vector.tensor_tensor(a0[:], m2, s[:], ALU.add)
                z0 = ctmp.tile([128, WT, WT], CONV_DT, tag="zk", bufs=8, name=f"z2e_{nh}_{mp}_{mm_}")
                nc.vector.tensor_tensor(z0[:], m0, a0[:], ALU.add)
                a1 = ctmp.tile([128, WT, WT], CONV_DT, tag="iv", bufs=6, name=f"a2b_{nh}_{mp}_{mm_}")
                nc.vector.tensor_tensor(a1[:], s[:], m2, ALU.subtract)
                z1 = ctmp.tile([128, WT, WT], CONV_DT, tag="zk", bufs=8, name=f"z2o_{nh}_{mp}_{mm_}")
                nc.vector.tensor_tensor(z1[:], a1[:], m3, ALU.subtract)
                zk[(mp, mm_, 0)] = z0
                zk[(mp, mm_, 1)] = z1
        # gating: out = relu((o2w + c2bw)*o1 + o2b + c2bb) per (mch, parity);
        # parity 0 = even orig w = split o1 cols 23:45, parity 1 = split 1:23
        for mch in range(2):
            for par in range(2):
                o1s = o1p_t[:, mch, 1 + r0:1 + r0 + WT,
                            slice(23, 45) if par == 0 else slice(1, 23)]
                t1 = ctmp.tile([128, WT, WT], F32, tag="g1", name=f"t1_{nh}_{mch}_{par}")
                nc.vector.scalar_tensor_tensor(
                    t1[:], zk[(0, mch, par)][:],
                    f32(prm_t[:, c2b_col + mch:c2b_col + mch + 1]),
                    o1s, ALU.add, ALU.mult)
                t2 = ctmp.tile([128, WT, WT], F32, tag="g2", name=f"t2_{nh}_{mch}_{par}")
                nc.vector.scalar_tensor_tensor(
                    t2[:], zk[(1, mch, par)][:],
                    f32(prm_t[:, c2b_col + mch + 2:c2b_col + mch + 3]),
                    t1[:], ALU.add, ALU.add)
                ov = out_t[:, mch, W * r0:W * (r0 + WT)].rearrange(
                    "p (r j t) -> p r j t", r=WT, t=2)
                nc.scalar.activation(ov[:, :, :, par], t2[:], AF.Relu)


def _att_weights(nc, qkw_d, vw_d, pool, tags):
    qkw_t = pool.tile([128, 2, 64], F32R, tag=tags + "qkw", name=tags + "qkw")
    vw_t = pool.tile([128, 2, 256], F32R, tag=tags + "vw", name=tags + "vw")
    for kc in range(2):
        nc.sync.dma_start(qkw_t[:, kc, :], qkw_d[kc].bitcast(F32R))
        nc.sync.dma_start(vw_t[:, kc, :], vw_d[kc].bitcast(F32R))
    return qkw_t, vw_t


def _att_qk_alloc(nc, prm_t, pool, tags):
    """Allocate q4 [128, N] / k4 [128, NK]: 4 replicated 32-row bands for the
    row-tiled score matmuls. k4 band-0 cols 1936:NK are zeroed (phantom n for
    jn=15: scores 0 -> exp 1 -> killed by vT zero rows; z uses the partial-ones
    col); replication DMAs propagate the zeros to bands 1-3."""
    zw = prm_t[:, C_ZEROW:C_ZEROW + PX]
    q4_t = pool.tile([128, N], BF16, tag=tags + "q", name=tags + "q")
    k4_t = pool.tile([128, NK], BF16, tag=tags + "k", name=tags + "k")
    nc.vector.tensor_copy(k4_t[0:32, N:NK], zw[0:32, 0:NK - N])
    return q4_t, k4_t


def _att_qk(nc, qkw_t, qb_col, kb_col, src_qk, prm_t, q4_t, k4_t, pspool, tags,
            ps_tag="cps", ps_bufs=None):
    """Fill q4/k4 band 0 from src_qk projections, then replicate bands via
    SBUF->SBUF DMA (no compute-engine cost; consumers are far downstream)."""
    f32 = lambda ap: ap.bitcast(F32)
    kw = dict(bufs=ps_bufs) if ps_bufs is not None else {}
    for im in range(NCH):
        msl = slice(PX * im, PX * (im + 1))
        pq = pspool.tile([64, PX], F32, tag=ps_tag, name=tags + f"pq{im}", **kw)
        for kc in range(2):
            _mm(nc, 'qk', pq[:], qkw_t[:, kc, :], src_qk[:, kc, msl],
                start=(kc == 0), stop=(kc == 1), skip_group_check=True)
        nc.vector.tensor_scalar_add(q4_t[0:32, msl], pq[0:32, :], f32(prm_t[0:32, qb_col:qb_col + 1]))
        nc.vector.tensor_scalar_add(k4_t[0:32, msl], pq[32:64, :], f32(prm_t[0:32, kb_col:kb_col + 1]))
    for t, w in ((q4_t, N), (k4_t, NK)):
        for b in range(1, 4):
            nc.sync.dma_start(t[32 * b:32 * b + 32, 0:w], t[0:32, 0:w])


def _att_v_alloc(nc, prm_t, pool, tags):
    zw = prm_t[:, C_ZEROW:C_ZEROW + PX]
    vT_t = pool.tile([128, AJ, 256], F32R, tag=tags + "vT", name=tags + "vT")
    nc.vector.tensor_copy(vT_t[:, AJ - 1, :], zw[:, 0:256])
    return vT_t


def _att_v_pairs(nc, vw_t, src_v, vT_t, pspool, tags, jps,
                 ps_tag="cps", ps_bufs=None, dve_only=False):
    """vT chunk pairs (2 jn per PSUM bank); rows 16:128 of the last chunk stay
    zeroed. Drains alternate scalar/vector unless dve_only (keeps ACT free)."""
    kw = dict(bufs=ps_bufs) if ps_bufs is not None else {}
    for jp in jps:
        tg = ps_tag[jp % len(ps_tag)] if isinstance(ps_tag, (list, tuple)) else ps_tag
        pv = pspool.tile([128, 2, 256], F32, tag=tg, name=tags + f"pv{jp}", **kw)
        for h in range(2):
            jn = 2 * jp + h
            nsz = 128 if jn < AJ - 1 else 16
            for kc in range(2):
                _mm(nc, 'vT', pv[0:nsz, h, :],
                    src_v[:, kc, 128 * jn:128 * jn + nsz],
                    vw_t[:, kc, :],
                    start=(kc == 0), stop=(kc == 1), skip_group_check=True)
        if jp < AJ // 2 - 1:
            if jp % 2 == 0 and not dve_only:
                nc.scalar.copy(vT_t[:, 2 * jp:2 * jp + 2, :], pv[:, :, :])
            else:
                nc.vector.tensor_copy(vT_t[:, 2 * jp:2 * jp + 2, :], pv[:, :, :])
        else:
            if dve_only:
                nc.vector.tensor_copy(vT_t[:, AJ - 2, :], pv[:, 0, :])
            else:
                nc.scalar.copy(vT_t[:, AJ - 2, :], pv[:, 0, :])
            nc.vector.tensor_copy(vT_t[0:16, AJ - 1, :], pv[0:16, 1, :])


def _att_pipeline(nc, atts, prm_t, aps, atmp, epool, prologue):
    """Pipelined attention over 8 chunks (2 atts x 4 m-blocks of 484).

    Per chunk: 4 ladder passes, each = 4 concurrently-executing row-tiled K=32
    score matmuls (tile_position bands) into 4 PSUM banks, drained by ONE wide
    exp into eT. Then 16 z + 32 feat matmuls (full array), normalize via
    approx-reciprocal + gpsimd broadcast, residual add, DMA out. Chunk c+1's
    ladder passes are interleaved with chunk c's z/feat groups so the PE never
    sits behind ACT exp; `prologue` callables fill chunk 0's exp-wait gaps.
    """
    f32 = lambda ap: ap.bitcast(F32)
    eTs, state = {}, {}

    def emit_z(c, jns):
        att, eT = atts[c // 4], eTs[c]
        for jn in jns:
            oc = C_ONESP if jn == AJ - 1 else C_ONESC
            _mm(nc, 'z', state[c]['pz'][0:1, :], prm_t[:, oc:oc + 1], eT[:, jn, :],
                start=(jn == 0), stop=(jn == AJ - 1), skip_group_check=True)

    def ladder_pass(c, p):
        att, im = atts[c // 4], c % 4
        msl = slice(PX * im, PX * (im + 1))
        if p == 0:
            eTs[c] = epool.tile([128, AJ, PX], F32R, tag="eT", name=f"eT{c}")
            state[c] = dict(
                pf=aps.tile([128, 2, 512], F32, tag="f", bufs=1, name=f"pf_{c}"),
                pz=aps.tile([1, PX], F32, tag="z", bufs=1, name=f"pz_{c}"))
        pst = aps.tile([128, 4, 512], F32, tag="st", bufs=1, name=f"pst_{c}_{p}")
        for r in range(4):
            jn = 4 * p + r
            _mm(nc, 'sT', pst[0:128, r, 0:PX],
                att['k4'][32 * r:32 * r + 32, 128 * jn:128 * jn + 128],
                att['q4'][32 * r:32 * r + 32, msl],
                start=True, stop=True, skip_group_check=True,
                tile_position=(32 * r, 0))
        nc.scalar.activation(eTs[c][:, 4 * p:4 * p + 4, :], pst[:, :, 0:PX], AF.Exp)
        if p >= 1:
            # z for the previous pass's (already exp'd) chunks: pz completes at
            # the start of chunk c's compute, so 1/Z + broadcast run a full
            # chunk ahead of the fo muls that consume them
            emit_z(c, range(4 * (p - 1), 4 * p))

    def group(c, g):
        att = atts[c // 4]
        eT = eTs[c]
        st = state[c]
        if g == 0:
            emit_z(c, range(AJ - 4, AJ))
            invz = atmp.tile([1, PX], F32, tag="invz", bufs=2, name=f"invz{c}")
            nc.vector.reciprocal_approx_fast(invz[0:1, :], st['pz'][0:1, :])
            izb = atmp.tile([128, PX], F32, tag="izb", bufs=2, name=f"izb{c}")
            nc.gpsimd.partition_broadcast(izb[:], invz[0:1, :])
            st['izb'] = izb
        if c == 7 and g == 3:
            # last chunk's last group: split feats by column half (first half
            # finishes early so its normalize/DMA chain hides under the second)
            for a, b in ((0, PX // 2), (PX // 2, PX)):
                for r in range(4):
                    jn = 4 * g + r
                    for h in range(2):
                        _mm(nc, 'feat', st['pf'][:, h, a:b],
                            att['vT'][:, jn, 128 * h:128 * h + 128], eT[:, jn, a:b],
                            start=False, stop=(jn == AJ - 1), skip_group_check=True)
        else:
            for r in range(4):
                jn = 4 * g + r
                for h in range(2):
                    _mm(nc, 'feat', st['pf'][:, h, 0:PX],
                        att['vT'][:, jn, 128 * h:128 * h + 128], eT[:, jn, :],
                        start=(jn == 0), stop=(jn == AJ - 1), skip_group_check=True)

    def post(c):
        att, im = atts[c // 4], c % 4
        st = state.pop(c)
        # last chunk: split into halves so DVE/DMA pipeline on the exposed tail
        spans = ((0, PX // 2), (PX // 2, PX)) if c == 7 else ((0, PX),)
        for cch in range(2):
            for a, b in spans:
                csl = slice(PX * im + a, PX * im + b)
                fo = atmp.tile([128, b - a], F32, tag="fo", bufs=4, name=f"fo{c}_{cch}{a}")
                nc.vector.tensor_mul(fo[:], st['pf'][:, cch, a:b], st['izb'][:, a:b])
                oo = atmp.tile([128, b - a], F32, tag="oo", bufs=4, name=f"oo{c}_{cch}{a}")
                # out = (feat/Z + vb) + r   (v-bias folded here: sum(mask)=1)
                nc.vector.scalar_tensor_tensor(
                    oo[:], fo[:], f32(prm_t[:, att['vb'] + cch:att['vb'] + cch + 1]),
                    f32(att['res'][:, cch, csl]), ALU.add, ALU.add)
                nc.sync.dma_start(att['out'][cch, :, csl], oo[:])
        eTs.pop(c)

    for p in range(4):
        ladder_pass(0, p)
        if p < len(prologue):
            prologue[p]()
    for c in range(8):
        for g in range(4):
            if c < 7:
                ladder_pass(c + 1, g)
            group(c, g)
        post(c)


def build_nc():
    nc = bacc.Bacc(None)
    d = {}
    cdt = CONV_DT if BF16_CONV else F32
    d['xr'] = nc.dram_tensor("xr", [4, 128, HP, WP], cdt, kind="ExternalInput")
    d['xd'] = nc.dram_tensor("xd", [4, 128, HP, WP], cdt, kind="ExternalInput")
    d['w1r'] = nc.dram_tensor("w1r", [4, 128, 12, 256], cdt, kind="ExternalInput")
    d['w2r'] = nc.dram_tensor("w2r", [2, 128, 12, 512], cdt, kind="ExternalInput")
    d['w1d'] = nc.dram_tensor("w1d", [4, 128, 12, 256], cdt, kind="ExternalInput")
    d['w2d'] = nc.dram_tensor("w2d", [2, 128, 12, 512], cdt, kind="ExternalInput")
    for a in (1, 2):
        d[f'qkw{a}'] = nc.dram_tensor(f"qkw{a}", [2, 128, 64], F32, kind="ExternalInput")
        d[f'vw{a}'] = nc.dram_tensor(f"vw{a}", [2, 128, 256], F32, kind="ExternalInput")
    d['prm'] = nc.dram_tensor("prm", [128, PRM_COLS], F32, kind="ExternalInput")
    d['o1'] = nc.dram_tensor("o1", [2, 128, N], F32, kind="ExternalOutput")
    d['o2'] = nc.dram_tensor("o2", [2, 128, N], F32, kind="ExternalOutput")

    with tile.TileContext(nc) as tc:
        with tc.tile_pool(name="persist", bufs=1) as persist, \
             tc.tile_pool(name="aearly", bufs=1) as aearly:
            prm_t = persist.tile([128, PRM_COLS], F32R, tag="prm")
            nc.sync.dma_start(prm_t[:], d['prm'][:].bitcast(F32R))
            r_t = persist.tile([128, 2, N], F32R, tag="r")
            d_t = persist.tile([128, 2, N], F32R, tag="d")

            with tc.tile_pool(name="wpool", bufs=3) as wpool, \
                 tc.tile_pool(name="xpool", bufs=3) as xpool, \
                 tc.tile_pool(name="o1pool", bufs=1) as o1pool, \
                 tc.tile_pool(name="cps", bufs=8, space="PSUM") as cps, \
                 tc.tile_pool(name="ctmp", bufs=3) as ctmp:
                o1p_t = o1pool.tile([128, 2, HP, WP], CONV_DT, tag="o1p")
                _conv_stream(nc, tc, d['xr'], d['w1r'], d['w2r'],
                             C_BNS1, C_BNT1, C_C2B1, prm_t, o1p_t, r_t,
                             wpool, xpool, cps, ctmp, True)
                # rgb-dependent attention preps run while depth convs stream:
                # att1 v comes from r, att2 q/k come from r
                qkw1_t, vw1_t = _att_weights(nc, d['qkw1'], d['vw1'], aearly, "a1")
                qkw2_t, vw2_t = _att_weights(nc, d['qkw2'], d['vw2'], aearly, "a2")
                q1_t, k1_t = _att_qk_alloc(nc, prm_t, aearly, "a1")
                q2_t, k2_t = _att_qk_alloc(nc, prm_t, aearly, "a2")
                vT1_t = _att_v_alloc(nc, prm_t, aearly, "a1")
                vT2_t = _att_v_alloc(nc, prm_t, aearly, "a2")
                _att_qk(nc, qkw2_t, C_QB2, C_KB2, r_t, prm_t, q2_t, k2_t, cps, "a2")
                _conv_stream(nc, tc, d['xd'], d['w1d'], d['w2d'],
                             C_BNS2, C_BNT2, C_C2B2, prm_t, o1p_t, d_t,
                             wpool, xpool, cps, ctmp, False)
                # vT1 last: its consumer (att1, chunks 4-7) is ~170us away, its
                # drains queued mid-stream were stalling the PE between convs,
                # and its matmuls now fill the psum-pool close barrier
                _att_v_pairs(nc, vw1_t, r_t, vT1_t, cps, "a1", range(AJ // 2))

            with tc.tile_pool(name="aps", bufs=2, space="PSUM") as aps, \
                 tc.tile_pool(name="atmp", bufs=2) as atmp, \
                 tc.tile_pool(name="epool", bufs=2) as epool:
                # att2 first: its q/k come from r_t (ready since mid-depth-conv), so
                # its score ladder starts immediately after the depth conv; att1's
                # q/k projections + vT2 fill chunk-0's exp-wait gaps (prologue),
                # and att1's band fills complete while att2 runs.
                atts = [
                    dict(q4=q2_t, k4=k2_t, vT=vT2_t, vb=C_VB2, res=d_t, out=d['o2']),
                    dict(q4=q1_t, k4=k1_t, vT=vT1_t, vb=C_VB1, res=r_t, out=d['o1']),
                ]
                prologue = [
                    lambda: _att_qk(nc, qkw1_t, C_QB1, C_KB1, d_t, prm_t,
                                    q1_t, k1_t, aps, "a1", ps_tag="pq", ps_bufs=1),
                    lambda: _att_v_pairs(nc, vw2_t, d_t, vT2_t, aps, "a2",
                                         range(0, AJ // 4), ps_tag=("f", "pq"),
                                         ps_bufs=1, dve_only=True),
                    lambda: _att_v_pairs(nc, vw2_t, d_t, vT2_t, aps, "a2",
                                         range(AJ // 4, AJ // 2), ps_tag=("f", "pq"),
                                         ps_bufs=1, dve_only=True),
                ]
                _att_pipeline(nc, atts, prm_t, aps, atmp, epool, prologue)

    nc.finalize()
    return nc


def _prep_common(g):
    """Host-side weight layout prep (shared across cores)."""
    out = {}
    for pre, kw1, kw2 in (('sa1', 'w1r', 'w2r'), ('sa2', 'w1d', 'w2d')):
        c1w = g[f'{pre}_c1_w']  # [256, 512, 3, 3]
        c2w = g[f'{pre}_c2_w']  # [512, 256, 3, 3]
        cnp = ml_dtypes.bfloat16 if BF16_CONV else np.float32
        # conv1: 1D-Winograd G-transform along dx: 3 taps -> 4 w-coords
        w = c1w.transpose(1, 2, 3, 0)  # [512, 3dy, 3dx, 256]
        wg = np.stack([w[:, :, 0], (w[:, :, 0] + w[:, :, 1] + w[:, :, 2]) * 0.5,
                       (w[:, :, 0] - w[:, :, 1] + w[:, :, 2]) * 0.5, w[:, :, 2]],
                      axis=2)  # [512, 3dy, 4wc, 256]
        out[kw1] = np.ascontiguousarray(wg.reshape(4, 128, 12, 256).astype(cnp))
        w2 = c2w.transpose(1, 2, 3, 0)  # [256, 3dy, 3dx, 512]
        wg2 = np.stack([w2[:, :, 0], (w2[:, :, 0] + w2[:, :, 1] + w2[:, :, 2]) * 0.5,
                        (w2[:, :, 0] - w2[:, :, 1] + w2[:, :, 2]) * 0.5, w2[:, :, 2]],
                       axis=2)  # [256, 3dy, 4wc, 512]
        out[kw2] = np.ascontiguousarray(wg2.reshape(2, 128, 12, 512).astype(cnp))

    gate = float(g['gate'][0]); beta = float(g['beta'][0]); gamma = float(g['gamma'][0])
    s1 = gate * beta
    s2 = (1.0 - gate) * gamma
    for a, s in ((1, s1), (2, s2)):
        vw = (s * g[f'a{a}_vw']).astype(np.float32)
        qkw = np.concatenate([g[f'a{a}_qw'], g[f'a{a}_kw']], axis=0)  # [64, 256]
        out[f'qkw{a}'] = np.ascontiguousarray(qkw.T.reshape(2, 128, 64))
        out[f'vw{a}'] = np.ascontiguousarray(vw.T.reshape(2, 128, 256))

    prm = np.zeros((128, PRM_COLS), np.float32)
    for pre, cs, ct, cb in (('sa1', C_BNS1, C_BNT1, C_C2B1), ('sa2', C_BNS2, C_BNT2, C_C2B2)):
        s = (g[f'{pre}_bn_g'] / np.sqrt(g[f'{pre}_bn_v'] + EPS)).astype(np.float32)
        t = ((g[f'{pre}_c1_b'] - g[f'{pre}_bn_m']) * s + g[f'{pre}_bn_b']).astype(np.float32)
        prm[:, cs:cs + 2] = s.reshape(2, 128).T
        prm[:, ct:ct + 2] = t.reshape(2, 128).T
        prm[:, cb:cb + 4] = g[f'{pre}_c2_b'].reshape(4, 128).T
    prm[0:32, C_QB1] = g['a1_qb']; prm[0:32, C_KB1] = g['a1_kb']
    prm[0:32, C_QB2] = g['a2_qb']; prm[0:32, C_KB2] = g['a2_kb']
    prm[:, C_VB1:C_VB1 + 2] = (s1 * g['a1_vb']).astype(np.float32).reshape(2, 128).T
    prm[:, C_VB2:C_VB2 + 2] = (s2 * g['a2_vb']).astype(np.float32).reshape(2, 128).T
    prm[0, C_ONESR:C_ONESR + 128] = 1.0
    prm[:, C_ONESC] = 1.0
    prm[0:16, C_ONESP] = 1.0   # z station for jn=15: ignore phantom rows 16:128
    out['prm'] = prm
    return out


def _prep_x(x):
    """[512, 44, 44] -> padded even/odd-split [4, 128, 46, 46]:
    cols 0:23 = padded even cols, 23:46 = padded odd cols."""
    p = np.zeros((512, HP, WP), np.float32)
    p[:, 1:45, 1:45] = x
    ps = np.concatenate([p[:, :, 0::2], p[:, :, 1::2]], axis=2)
    return ps.reshape(4, 128, HP, WP).astype(
        ml_dtypes.bfloat16 if BF16_CONV else np.float32)


_NC_CACHE = None


def kernel(**inputs):
    global _NC_CACHE
    g = {k: np.asarray(v, np.float32) for k, v in inputs.items()}
    if _NC_CACHE is None:
        _NC_CACHE = build_nc()
    nc = _NC_CACHE

    common = _prep_common(g)
    B = g['rgb'].shape[0]
    in_maps = []
    for b in range(B):
        m = dict(common)
        m['xr'] = _prep_x(g['rgb'][b])
        m['xd'] = _prep_x(g['depth'][b])
        in_maps.append(m)

    res = run_bass_kernel_spmd(nc, in_maps, list(range(B)))
    out1 = np.stack([res.results[b]['o1'].reshape(256, H, W) for b in range(B)])
    out2 = np.stack([res.results[b]['o2'].reshape(256, H, W) for b in range(B)])
    return out1, out2

